# revision 1
# baseline (speedup 1.0000x reference)
"""Trainium2 Bass kernel: out = x @ ((W_int + offset) * scale), fp8 DoubleRow.

Math: V = W - 63 (zero-mean ints, |V| <= 63), cast to fp8 e4m3 (E[dV^2]~0.88);
x cast to fp8 e4m3 (rms rel err ~2.6%). Then
  out[m,n] = scale[n] * ((x8 @ V8)[m,n] + (63 + offset[n]) * rowsum(x)[m])
The rank-1 term uses the exact f32 rowsum, so W's mean and the offset are
exact; total rel err ~1.84e-2 (measured vs f64 on the actual inputs).

PE: DoubleRow fp8 matmuls (2 fp8 per cell along K, 2x MACs/cycle).
Orientation: W is stationary ([128k, 2slot, 128n] per (kp, nb)), x^T is
moving ([128k, 2slot, 512m] chunks) -> output transposed [n, m] in PSUM.
Each (nb, m-quarter) group accumulates 16 k-pairs x 2 half-chunks = 32 MMs
into a [128, 1024] PSUM tile; epilogue: DVE adds the rank-1 term
(s_bcast * offc), ACT applies the per-n scale to SBUF, DMA to a transposed
DRAM output [1408, 4096] (host un-transposes and crops).

Sharding: column-parallel over N across 8 cores (NSH=1376 each).
Warmup: first 4 nb-groups of quarter 0 run kp-interleaved so the PE tracks
W/x DMA arrival; x quarters are double-buffered, W (5.5MB fp8) persistent.

Measured: 337.5-339.5us HW exec (8 cores) vs 623.7us for the bf16 rank-1
epilogue baseline (1.85x); MATMUL stream 305.7us at ~221ns per 512-col
DoubleRow MM, LDWEIGHTS fully hidden. Rel err 1.8449e-2 vs f64 reference
(threshold 2e-2, deterministic seeded inputs; verified by exact numpy
simulation of the quantization in err_sim.py).
"""

import numpy as np
import ml_dtypes

M, K, N = 4096, 4096, 11008
NCORES = 8
NSH = N // NCORES          # 1376
P = 128
KP = 16                    # k-pairs of 256
NB = 11                    # n-blocks: 10x128 + 96
NBW = [128] * 10 + [96]
NQ = 4                     # m-quarters
MQW = 1024                 # m-quarter width
NBPAD = NB * P             # 1408 padded rows of transposed out

_E4 = ml_dtypes.float8_e4m3

_cache = {}


def _build_nc():
    import concourse.bacc as bacc
    import concourse.mybir as mybir
    import concourse.tile as tile

    fp8 = mybir.dt.float8e4
    f32 = mybir.dt.float32
    DR = mybir.MatmulPerfMode.DoubleRow
    Copy = mybir.ActivationFunctionType.Copy

    nc = bacc.Bacc(None, target_bir_lowering=False)
    # xq rows: (q*KP + kp)*P + p ; cols: slot*MQW + m  (x^T in fp8 pairs)
    xq = nc.dram_tensor("xq", [NQ * KP * P, 2 * MQW], fp8, kind="ExternalInput")
    # wq rows: kp*P + p ; cols: slot*NSH + n  (V8 = e4m3(W-63) pairs)
    wq = nc.dram_tensor("wq", [KP * P, 2 * NSH], fp8, kind="ExternalInput")
    sbc = nc.dram_tensor("sbc", [P, M], f32, kind="ExternalInput")      # rowsum bcast
    offc = nc.dram_tensor("offc", [P, NB], f32, kind="ExternalInput")   # 63+offset
    scalec = nc.dram_tensor("scalec", [P, NB], f32, kind="ExternalInput")
    outt = nc.dram_tensor("outt", [NBPAD, M], f32, kind="ExternalOutput")

    xq3 = xq.ap().rearrange("(g p) f -> p g f", p=P)        # [128, NQ*KP, 2048]
    wq3 = wq.ap().rearrange("(kp p) f -> p kp f", p=P)      # [128, KP, 2752]
    outt3 = outt.ap().rearrange("(nb p) m -> p nb m", p=P)  # [128, NB, 4096]

    with tile.TileContext(nc) as tc:
        with (
            tc.tile_pool(name="wpool", bufs=KP) as wpool,
            tc.tile_pool(name="xpool", bufs=2 * KP) as xpool,
            tc.tile_pool(name="cpool", bufs=1) as cpool,
            tc.tile_pool(name="opool", bufs=4) as opool,
            tc.tile_pool(name="psp", bufs=4, space="PSUM") as psp,
        ):
            # W: all 16 kp tiles, persistent. even->gpsimd, odd->scalar.
            w_sb = []
            for kp in range(KP):
                t = wpool.tile([P, 2, NSH], fp8, tag="w", name=f"w{kp}")
                eng = nc.gpsimd if kp % 2 == 0 else nc.scalar
                eng.dma_start(
                    t[:], wq3[:, kp, :].rearrange("p (s n) -> p s n", s=2)
                )
                w_sb.append(t)

            x_tiles = {}

            def load_xq(q):
                for kp in range(KP):
                    t = xpool.tile([P, 2, MQW], fp8, tag="x", name=f"x{q}_{kp}")
                    nc.sync.dma_start(
                        t[:],
                        xq3[:, q * KP + kp, :].rearrange(
                            "p (s m) -> p s m", s=2
                        ),
                    )
                    x_tiles[(q, kp)] = t

            load_xq(0)

            # constants: rowsum-bcast split per quarter (q0 needed first)
            sbc_sb = cpool.tile([P, M], f32, tag="sbc")
            nc.sync.dma_start(sbc_sb[:, 0:MQW], sbc.ap()[:, 0:MQW])
            offc_sb = cpool.tile([P, NB], f32, tag="offc")
            nc.gpsimd.dma_start(offc_sb[:], offc.ap())
            scalec_sb = cpool.tile([P, NB], f32, tag="scalec")
            nc.gpsimd.dma_start(scalec_sb[:], scalec.ap())

            def mm(ps, q, nb, kp):
                nbw = NBW[nb]
                for h in range(2):
                    nc.tensor.matmul(
                        ps[:nbw, h * 512:(h + 1) * 512],
                        w_sb[kp][:, :, nb * P:nb * P + nbw],
                        x_tiles[(q, kp)][:, :, h * 512:(h + 1) * 512],
                        start=(kp == 0),
                        stop=(kp == KP - 1),
                        perf_mode=DR,
                    )

            def epilogue(ps, q, nb):
                nbw = NBW[nb]
                # ps += (63 + offset[n]) * rowsum_x[m]
                nc.vector.scalar_tensor_tensor(
                    ps[:nbw, :],
                    sbc_sb[:nbw, q * MQW:(q + 1) * MQW],
                    offc_sb[:nbw, nb:nb + 1],
                    ps[:nbw, :],
                    mybir.AluOpType.mult,
                    mybir.AluOpType.add,
                )
                o_sb = opool.tile([P, MQW], f32, tag="o")
                # o = ps * scale[n]
                nc.scalar.activation(
                    o_sb[:nbw, :],
                    ps[:nbw, :],
                    Copy,
                    scale=scalec_sb[:nbw, nb:nb + 1],
                )
                nc.gpsimd.dma_start(
                    outt3[:nbw, nb, q * MQW:(q + 1) * MQW], o_sb[:nbw, :]
                )

            # Phase A: quarter 0, nb 0..3 kp-interleaved (tracks DMA arrival).
            ps_a = [
                psp.tile([P, MQW], f32, tag="ps", name=f"psA{g}")
                for g in range(4)
            ]
            for kp in range(KP):
                for g in range(4):
                    mm(ps_a[g], 0, g, kp)
            for g in range(4):
                epilogue(ps_a[g], 0, g)

            # Phase B/C: remaining groups, x quarters prefetched one ahead.
            for q in range(NQ):
                nb0 = 4 if q == 0 else 0
                for nb in range(nb0, NB):
                    if q + 1 < NQ and nb == nb0:
                        load_xq(q + 1)
                        if q == 0:
                            nc.sync.dma_start(
                                sbc_sb[:, MQW:], sbc.ap()[:, MQW:]
                            )
                    ps = psp.tile([P, MQW], f32, tag="ps")
                    for kp in range(KP):
                        mm(ps, q, nb, kp)
                    epilogue(ps, q, nb)
    nc.compile()
    return nc


def _get_nc():
    if "nc" not in _cache:
        _cache["nc"] = _build_nc()
    return _cache["nc"]


def _prep_inputs(x, weight, antiquant_scale, antiquant_offset):
    x = np.asarray(x, dtype=np.float32)
    weight = np.asarray(weight)
    scale = np.asarray(antiquant_scale, dtype=np.float32)
    off = np.asarray(antiquant_offset, dtype=np.float32)

    x8t = x.astype(_E4).T                      # [K, M] fp8
    xdr = np.ascontiguousarray(
        x8t.reshape(KP, 2, P, NQ, MQW).transpose(3, 0, 2, 1, 4)
    ).reshape(NQ * KP * P, 2 * MQW)
    rs = x.astype(np.float64).sum(axis=1).astype(np.float32)
    sbc = np.ascontiguousarray(np.broadcast_to(rs[None, :], (P, M)))

    V8 = (weight.astype(np.float32) - 63.0).astype(_E4)   # [K, N]

    in_maps = []
    for c in range(NCORES):
        sl = slice(c * NSH, (c + 1) * NSH)
        wdr = np.ascontiguousarray(
            V8[:, sl].reshape(KP, 2, P, NSH).transpose(0, 2, 1, 3)
        ).reshape(KP * P, 2 * NSH)
        opad = np.zeros(NBPAD, dtype=np.float32)
        opad[:NSH] = 63.0 + off[sl]
        spad = np.zeros(NBPAD, dtype=np.float32)
        spad[:NSH] = scale[sl]
        in_maps.append({
            "xq": xdr,
            "wq": wdr,
            "sbc": sbc,
            "offc": np.ascontiguousarray(opad.reshape(NB, P).T),
            "scalec": np.ascontiguousarray(spad.reshape(NB, P).T),
        })
    return in_maps


def kernel(x, weight, antiquant_scale, antiquant_offset, _trace=False):
    from concourse.bass_utils import run_bass_kernel_spmd

    nc = _get_nc()
    in_maps = _prep_inputs(x, weight, antiquant_scale, antiquant_offset)
    res = run_bass_kernel_spmd(
        nc, in_maps, core_ids=list(range(NCORES)), trace=_trace
    )
    out = np.concatenate(
        [np.asarray(res.results[c]["outt"])[:NSH] for c in range(NCORES)],
        axis=0,
    )
    if _trace:
        _cache["last_result"] = res
    return np.ascontiguousarray(out.T).astype(np.float32)



# revision 3
# speedup vs baseline: 1.0153x; 1.0153x over previous
"""Trainium2 Bass kernel: out = x @ ((W_int + offset) * scale), fp8 DoubleRow.

Math (same quantization as the 343us baseline): V = W - 63 (|V| <= 63), cast
to fp8 e4m3; x cast to fp8 e4m3. Then
  out[m,n] = scale[n] * ((x8 @ V8)[m,n] + (63 + offset[n]) * rowsum(x)[m])
with the rank-1 term in exact f32 (DVE), so only x/V carry fp8 error.
Measured rel err ~1.845e-2 vs f64 (threshold 2e-2).

v2 changes vs the N-sharded baseline (338-343us):
- Shard along M instead of N: each core owns m-cols [c*512, (c+1)*512) and
  ALL of N. N = 11008 = 86*128 exactly, so there are no padded stationary
  blocks: 86 nb * 16 kp = 1376 DoubleRow MMs per core (vs 1408 padded),
  a 2.3% shorter PE stream. W (45MB fp8) is streamed per-nb (512KB tiles,
  148 GB/s sustained) instead of kept resident.
- Warmup: first W/x DMAs are issued as small pieces spread over 4 queues
  (scalar/sync/vector) so the first real MM starts ~9us instead of ~13us;
  ~16 dummy DoubleRow MMs on memset tiles keep the PE busy from ~6.5us so
  the HAM clock-gate reaches K=8/8 before the real stream begins (baseline
  ran at 1.2GHz until 30us, ~10us penalty).
- Output staged in bf16 (halves out DMA; adds ~0.1% RMS rounding, total err
  budget unaffected), epilogue per nb is [128,512] so the post-last-MM tail
  chain is ~2us instead of ~4us.

Per-core schedule: for nb in 0..85: 16 kp MMs accumulate into one PSUM bank
([128n, 512m], moving x chunk [128,2,512]); epilogue: DVE STT adds
rowsum*(63+off), ACT applies scale -> bf16 SBUF, gpsimd DMA to DRAM
outt[(nb p), m]. Host un-transposes and concatenates the 8 m-slices.
"""

import numpy as np
import ml_dtypes

M, K, N = 4096, 4096, 11008
NCORES = 8
MSH = M // NCORES          # 512 m-cols per core
P = 128
KP = 16                    # k-pairs of 256
NB = N // P                # 86 n-blocks, exact
WBUFS = 12                 # streamed W tiles in flight
NWARM = 16                 # dummy HAM-warmup matmuls

_E4 = ml_dtypes.float8_e4m3

_cache = {}


def _build_nc():
    import concourse.bacc as bacc
    import concourse.mybir as mybir
    import concourse.tile as tile

    fp8 = mybir.dt.float8e4
    f32 = mybir.dt.float32
    bf16 = mybir.dt.bfloat16
    DR = mybir.MatmulPerfMode.DoubleRow
    Copy = mybir.ActivationFunctionType.Copy

    nc = bacc.Bacc(None, target_bir_lowering=False)
    # xq rows: kp*P + p ; cols: slot*MSH + m  (x^T fp8 pairs, this core's m)
    xq = nc.dram_tensor("xq", [KP * P, 2 * MSH], fp8, kind="ExternalInput")
    # wq rows: nb*P + p ; cols: kp*256 + slot*128 + nn  (full W, fp8 pairs)
    wq = nc.dram_tensor("wq", [NB * P, KP * 2 * P], fp8, kind="ExternalInput")
    sbc = nc.dram_tensor("sbc", [P, MSH], f32, kind="ExternalInput")
    offc = nc.dram_tensor("offc", [P, NB], f32, kind="ExternalInput")
    scalec = nc.dram_tensor("scalec", [P, NB], f32, kind="ExternalInput")
    outt = nc.dram_tensor("outt", [NB * P, MSH], bf16, kind="ExternalOutput")

    xq3 = xq.ap().rearrange("(kp p) f -> p kp f", p=P)     # [128, 16, 1024]
    wq3 = wq.ap().rearrange("(nb p) f -> p nb f", p=P)     # [128, 86, 4096]
    outt3 = outt.ap().rearrange("(nb p) m -> p nb m", p=P)  # [128, 86, 512]

    with tile.TileContext(nc) as tc:
        with (
            tc.tile_pool(name="wpool", bufs=WBUFS) as wpool,
            tc.tile_pool(name="xpool", bufs=1) as xpool,
            tc.tile_pool(name="cpool", bufs=1) as cpool,
            tc.tile_pool(name="opool", bufs=3) as opool,
            tc.tile_pool(name="psp", bufs=4, space="PSUM") as psp,
            tc.tile_pool(name="pswarm", bufs=1, space="PSUM") as pswarm,
        ):
            # --- HAM warmup: memset junk tiles, dummy MMs keep PE busy ---
            wm = cpool.tile([P, 2, P], fp8, tag="wm")
            xm = cpool.tile([P, 2, P], fp8, tag="xm")
            nc.gpsimd.memset(wm[:], 0)
            nc.gpsimd.memset(xm[:], 0)
            psw = pswarm.tile([P, P], f32, tag="psw")
            for _ in range(NWARM):
                nc.tensor.matmul(
                    psw[:], wm[:], xm[:], start=True, stop=True, perf_mode=DR
                )

            # --- first-wave DMAs: small pieces, 4 queues ---
            w_sb = []
            t0 = wpool.tile([P, KP, 2, P], fp8, tag="w", name="w0")
            for q in range(4):  # w0 in 4 kp-quarters on scalar
                nc.scalar.dma_start(
                    t0[:, 4 * q:4 * q + 4, :, :],
                    wq3[:, 0, 1024 * q:1024 * (q + 1)].rearrange(
                        "p (k s n) -> p k s n", k=4, s=2
                    ),
                )
            w_sb.append(t0)
            t1 = wpool.tile([P, KP, 2, P], fp8, tag="w", name="w1")
            for h in range(2):  # w1 in 2 halves on scalar
                nc.scalar.dma_start(
                    t1[:, 8 * h:8 * h + 8, :, :],
                    wq3[:, 1, 2048 * h:2048 * (h + 1)].rearrange(
                        "p (k s n) -> p k s n", k=8, s=2
                    ),
                )
            w_sb.append(t1)

            # x: 4 persistent chunks of 4 kp each; kp0..7 split small on sync,
            # kp8..15 as two 512KB pieces on vector.
            x_sb = []
            for c in range(4):
                x_sb.append(
                    xpool.tile([P, 4, 2, MSH], fp8, tag=f"xc{c}", name=f"x{c}")
                )

            def load_x(c, k0, nk, eng):
                eng.dma_start(
                    x_sb[c][:, k0:k0 + nk, :, :],
                    xq3[:, 4 * c + k0:4 * c + k0 + nk, :].rearrange(
                        "p k (s m) -> p k s m", s=2
                    ),
                )

            load_x(0, 0, 1, nc.sync)    # kp0 first, smallest latency
            load_x(0, 1, 1, nc.sync)    # kp1
            load_x(0, 2, 2, nc.sync)    # kp2-3
            load_x(1, 0, 4, nc.gpsimd)  # kp4-7
            load_x(2, 0, 4, nc.gpsimd)  # kp8-11
            load_x(3, 0, 4, nc.gpsimd)  # kp12-15

            # constants on scalar (needed by first epilogue ~13us)
            sbc_sb = cpool.tile([P, MSH], f32, tag="sbc")
            nc.scalar.dma_start(sbc_sb[:], sbc.ap())
            offc_sb = cpool.tile([P, NB], f32, tag="offc")
            nc.scalar.dma_start(offc_sb[:], offc.ap())
            scalec_sb = cpool.tile([P, NB], f32, tag="scalec")
            nc.scalar.dma_start(scalec_sb[:], scalec.ap())

            # remaining W stream on sync, paced by wpool buf releases
            def load_w(nb):
                t = wpool.tile([P, KP, 2, P], fp8, tag="w", name=f"w{nb}")
                nc.sync.dma_start(
                    t[:],
                    wq3[:, nb, :].rearrange("p (k s n) -> p k s n", k=KP, s=2),
                )
                w_sb.append(t)

            for nb in range(2, WBUFS):
                load_w(nb)

            # --- main loop: 86 nb groups ---
            for nb in range(NB):
                if nb + WBUFS < NB:
                    load_w(nb + WBUFS)
                w = w_sb[nb]
                ps = psp.tile([P, MSH], f32, tag="ps")
                for kp in range(KP):
                    nc.tensor.matmul(
                        ps[:],
                        w[:, kp, :, :],
                        x_sb[kp // 4][:, kp % 4, :, :],
                        start=(kp == 0),
                        stop=(kp == KP - 1),
                        perf_mode=DR,
                    )
                # ps += (63 + offset[n]) * rowsum_x[m]
                nc.vector.scalar_tensor_tensor(
                    ps[:],
                    sbc_sb[:],
                    offc_sb[:, nb:nb + 1],
                    ps[:],
                    mybir.AluOpType.mult,
                    mybir.AluOpType.add,
                )
                o_sb = opool.tile([P, MSH], bf16, tag="o")
                nc.scalar.activation(
                    o_sb[:], ps[:], Copy, scale=scalec_sb[:, nb:nb + 1]
                )
                nc.gpsimd.dma_start(outt3[:, nb, :], o_sb[:])
    nc.compile()
    return nc


def _get_nc():
    if "nc" not in _cache:
        _cache["nc"] = _build_nc()
    return _cache["nc"]


def _prep_inputs(x, weight, antiquant_scale, antiquant_offset):
    x = np.asarray(x, dtype=np.float32)
    weight = np.asarray(weight)
    scale = np.asarray(antiquant_scale, dtype=np.float32)
    off = np.asarray(antiquant_offset, dtype=np.float32)

    xt8 = np.ascontiguousarray(x.astype(_E4).T)         # [K, M] fp8
    rs = x.astype(np.float64).sum(axis=1).astype(np.float32)

    V8 = (weight.astype(np.float32) - 63.0).astype(_E4)  # [K, N]
    # rows (nb, p), cols (kp, slot, nn); k = kp*256 + slot*128 + p
    wdr = np.ascontiguousarray(
        V8.reshape(KP, 2, P, NB, P).transpose(3, 2, 0, 1, 4)
    ).reshape(NB * P, KP * 2 * P)
    offc = np.ascontiguousarray((63.0 + off).reshape(NB, P).T)
    scalec = np.ascontiguousarray(scale.reshape(NB, P).T)

    in_maps = []
    for c in range(NCORES):
        sl = slice(c * MSH, (c + 1) * MSH)
        xdr = np.ascontiguousarray(
            xt8[:, sl].reshape(KP, 2, P, MSH).transpose(0, 2, 1, 3)
        ).reshape(KP * P, 2 * MSH)
        sbc = np.ascontiguousarray(
            np.broadcast_to(rs[sl][None, :], (P, MSH))
        )
        in_maps.append({
            "xq": xdr,
            "wq": wdr,
            "sbc": sbc,
            "offc": offc,
            "scalec": scalec,
        })
    return in_maps


def kernel(x, weight, antiquant_scale, antiquant_offset, _trace=False):
    from concourse.bass_utils import run_bass_kernel_spmd

    nc = _get_nc()
    in_maps = _prep_inputs(x, weight, antiquant_scale, antiquant_offset)
    res = run_bass_kernel_spmd(
        nc, in_maps, core_ids=list(range(NCORES)), trace=_trace
    )
    out = np.empty((M, N), dtype=np.float32)
    for c in range(NCORES):
        outt = np.asarray(res.results[c]["outt"])      # [N, MSH] bf16
        out[c * MSH:(c + 1) * MSH, :] = outt.T.astype(np.float32)
    if _trace:
        _cache["last_result"] = res
    return out


# revision 6
# speedup vs baseline: 1.0553x; 1.0394x over previous
"""Trainium2 Bass kernel: out = x @ ((W_int + offset) * scale), fp8 DoubleRow.

Math (same quantization as the 343us baseline): V = W - 63 (|V| <= 63), cast
to fp8 e4m3; x cast to fp8 e4m3. Then
  out[m,n] = scale[n] * ((x8 @ V8)[m,n] + (63 + offset[n]) * rowsum(x)[m])
with the rank-1 term in exact f32 (DVE), so only x/V carry fp8 error.
Measured rel err ~1.845e-2 vs f64 (threshold 2e-2).

v2 changes vs the N-sharded baseline (338-343us):
- Shard along M instead of N: each core owns m-cols [c*512, (c+1)*512) and
  ALL of N. N = 11008 = 86*128 exactly, so there are no padded stationary
  blocks: 86 nb * 16 kp = 1376 DoubleRow MMs per core (vs 1408 padded),
  a 2.3% shorter PE stream. W (45MB fp8) is streamed per-nb (512KB tiles,
  148 GB/s sustained) instead of kept resident.
- Warmup: first W/x DMAs are issued as small pieces spread over 4 queues
  (scalar/sync/vector) so the first real MM starts ~9us instead of ~13us;
  ~16 dummy DoubleRow MMs on memset tiles keep the PE busy from ~6.5us so
  the HAM clock-gate reaches K=8/8 before the real stream begins (baseline
  ran at 1.2GHz until 30us, ~10us penalty).
- Output staged in bf16 (halves out DMA; adds ~0.1% RMS rounding, total err
  budget unaffected), epilogue per nb is [128,512] so the post-last-MM tail
  chain is ~2us instead of ~4us.

Per-core schedule: for nb in 0..85: 16 kp MMs accumulate into one PSUM bank
([128n, 512m], moving x chunk [128,2,512]); epilogue: DVE STT adds
rowsum*(63+off), ACT applies scale -> bf16 SBUF, gpsimd DMA to DRAM
outt[(nb p), m]. Host un-transposes and concatenates the 8 m-slices.
"""

import numpy as np
import ml_dtypes

M, K, N = 4096, 4096, 11008
NCORES = 8
MSH = M // NCORES          # 512 m-cols per core
P = 128
KP = 16                    # k-pairs of 256
NB = N // P                # 86 n-blocks, exact
WBUFS = 8                  # streamed W tiles in flight
NWARM = 16                 # dummy HAM-warmup matmuls

_E4 = ml_dtypes.float8_e4m3

_cache = {}


def _build_nc():
    import concourse.bacc as bacc
    import concourse.mybir as mybir
    import concourse.tile as tile

    fp8 = mybir.dt.float8e4
    f32 = mybir.dt.float32
    bf16 = mybir.dt.bfloat16
    DR = mybir.MatmulPerfMode.DoubleRow
    Copy = mybir.ActivationFunctionType.Copy

    nc = bacc.Bacc(None, target_bir_lowering=False)
    # xq rows: kp*P + p ; cols: slot*MSH + m  (x^T fp8 pairs, this core's m)
    xq = nc.dram_tensor("xq", [KP * P, 2 * MSH], fp8, kind="ExternalInput")
    # wq rows: nb*P + p ; cols: kp*256 + slot*128 + nn  (full W, fp8 pairs)
    wq = nc.dram_tensor("wq", [NB * P, KP * 2 * P], fp8, kind="ExternalInput")
    sbc = nc.dram_tensor("sbc", [P, MSH], f32, kind="ExternalInput")
    offc = nc.dram_tensor("offc", [P, NB], f32, kind="ExternalInput")
    scalec = nc.dram_tensor("scalec", [P, NB], f32, kind="ExternalInput")
    outt = nc.dram_tensor("outt", [NB * P, MSH], bf16, kind="ExternalOutput")

    xq3 = xq.ap().rearrange("(kp p) f -> p kp f", p=P)     # [128, 16, 1024]
    wq3 = wq.ap().rearrange("(nb p) f -> p nb f", p=P)     # [128, 86, 4096]
    outt3 = outt.ap().rearrange("(nb p) m -> p nb m", p=P)  # [128, 86, 512]

    with tile.TileContext(nc) as tc:
        with (
            tc.tile_pool(name="wpool", bufs=WBUFS) as wpool,
            tc.tile_pool(name="xpool", bufs=1) as xpool,
            tc.tile_pool(name="cpool", bufs=1) as cpool,
            tc.tile_pool(name="opool", bufs=3) as opool,
            tc.tile_pool(name="psp", bufs=4, space="PSUM") as psp,
            tc.tile_pool(name="pswarm", bufs=1, space="PSUM") as pswarm,
        ):
            # --- HAM warmup: memset junk tiles, dummy MMs keep PE busy ---
            wm = cpool.tile([P, 2, P], fp8, tag="wm")
            xm = cpool.tile([P, 2, P], fp8, tag="xm")
            nc.gpsimd.memset(wm[:], 0)
            nc.gpsimd.memset(xm[:], 0)
            psw = pswarm.tile([P, P], f32, tag="psw")
            for _ in range(NWARM):
                nc.tensor.matmul(
                    psw[:], wm[:], xm[:], start=True, stop=True, perf_mode=DR
                )

            # --- first-wave DMAs: small pieces, 4 queues ---
            w_sb = []
            t0 = wpool.tile([P, KP, 2, P], fp8, tag="w", name="w0")
            for q in range(4):  # w0 in 4 kp-quarters on scalar
                nc.scalar.dma_start(
                    t0[:, 4 * q:4 * q + 4, :, :],
                    wq3[:, 0, 1024 * q:1024 * (q + 1)].rearrange(
                        "p (k s n) -> p k s n", k=4, s=2
                    ),
                )
            w_sb.append(t0)
            t1 = wpool.tile([P, KP, 2, P], fp8, tag="w", name="w1")
            for h in range(2):  # w1 in 2 halves on scalar
                nc.scalar.dma_start(
                    t1[:, 8 * h:8 * h + 8, :, :],
                    wq3[:, 1, 2048 * h:2048 * (h + 1)].rearrange(
                        "p (k s n) -> p k s n", k=8, s=2
                    ),
                )
            w_sb.append(t1)

            # x: 4 persistent chunks of 4 kp each; kp0..7 split small on sync,
            # kp8..15 as two 512KB pieces on vector.
            x_sb = []
            for c in range(4):
                x_sb.append(
                    xpool.tile([P, 4, 2, MSH], fp8, tag=f"xc{c}", name=f"x{c}")
                )

            def load_x(c, k0, nk, eng):
                eng.dma_start(
                    x_sb[c][:, k0:k0 + nk, :, :],
                    xq3[:, 4 * c + k0:4 * c + k0 + nk, :].rearrange(
                        "p k (s m) -> p k s m", s=2
                    ),
                )

            # x pieces sized so HWDGE arrival tracks the cold-MM consumption
            load_x(0, 0, 1, nc.sync)    # kp0 first, smallest latency
            load_x(0, 1, 1, nc.sync)    # kp1
            load_x(0, 2, 2, nc.sync)    # kp2-3
            load_x(1, 0, 2, nc.sync)    # kp4-5
            load_x(1, 2, 2, nc.sync)    # kp6-7
            load_x(2, 0, 2, nc.sync)    # kp8-9
            load_x(2, 2, 2, nc.sync)    # kp10-11
            load_x(3, 0, 2, nc.sync)    # kp12-13
            load_x(3, 2, 2, nc.sync)    # kp14-15

            # constants on scalar (needed by first epilogue ~13us)
            sbc_sb = cpool.tile([P, MSH], f32, tag="sbc")
            nc.scalar.dma_start(sbc_sb[:], sbc.ap())
            offc_sb = cpool.tile([P, NB], f32, tag="offc")
            nc.scalar.dma_start(offc_sb[:], offc.ap())
            scalec_sb = cpool.tile([P, NB], f32, tag="scalec")
            nc.scalar.dma_start(scalec_sb[:], scalec.ap())

            # remaining W stream on sync, paced by wpool buf releases
            def load_w(nb):
                t = wpool.tile([P, KP, 2, P], fp8, tag="w", name=f"w{nb}")
                nc.sync.dma_start(
                    t[:],
                    wq3[:, nb, :].rearrange("p (k s n) -> p k s n", k=KP, s=2),
                )
                w_sb.append(t)

            for nb in range(2, WBUFS):
                load_w(nb)

            # --- main loop: 86 nb groups ---
            for nb in range(NB):
                if nb + WBUFS < NB:
                    load_w(nb + WBUFS)
                w = w_sb[nb]
                ps = psp.tile([P, MSH], f32, tag="ps")
                for kp in range(KP):
                    nc.tensor.matmul(
                        ps[:],
                        w[:, kp, :, :],
                        x_sb[kp // 4][:, kp % 4, :, :],
                        start=(kp == 0),
                        stop=(kp == KP - 1),
                        perf_mode=DR,
                    )
                # ps += (63 + offset[n]) * rowsum_x[m]
                nc.vector.scalar_tensor_tensor(
                    ps[:],
                    sbc_sb[:],
                    offc_sb[:, nb:nb + 1],
                    ps[:],
                    mybir.AluOpType.mult,
                    mybir.AluOpType.add,
                )
                o_sb = opool.tile([P, MSH], bf16, tag="o")
                nc.scalar.activation(
                    o_sb[:], ps[:], Copy, scale=scalec_sb[:, nb:nb + 1]
                )
                nc.scalar.dma_start(outt3[:, nb, :], o_sb[:])
    nc.compile()
    return nc


def _get_nc():
    if "nc" not in _cache:
        _cache["nc"] = _build_nc()
    return _cache["nc"]


def _prep_inputs(x, weight, antiquant_scale, antiquant_offset):
    x = np.asarray(x, dtype=np.float32)
    weight = np.asarray(weight)
    scale = np.asarray(antiquant_scale, dtype=np.float32)
    off = np.asarray(antiquant_offset, dtype=np.float32)

    xt8 = np.ascontiguousarray(x.astype(_E4).T)         # [K, M] fp8
    rs = x.astype(np.float64).sum(axis=1).astype(np.float32)

    V8 = (weight.astype(np.float32) - 63.0).astype(_E4)  # [K, N]
    # rows (nb, p), cols (kp, slot, nn); k = kp*256 + slot*128 + p
    wdr = np.ascontiguousarray(
        V8.reshape(KP, 2, P, NB, P).transpose(3, 2, 0, 1, 4)
    ).reshape(NB * P, KP * 2 * P)
    offc = np.ascontiguousarray((63.0 + off).reshape(NB, P).T)
    scalec = np.ascontiguousarray(scale.reshape(NB, P).T)

    in_maps = []
    for c in range(NCORES):
        sl = slice(c * MSH, (c + 1) * MSH)
        xdr = np.ascontiguousarray(
            xt8[:, sl].reshape(KP, 2, P, MSH).transpose(0, 2, 1, 3)
        ).reshape(KP * P, 2 * MSH)
        sbc = np.ascontiguousarray(
            np.broadcast_to(rs[sl][None, :], (P, MSH))
        )
        in_maps.append({
            "xq": xdr,
            "wq": wdr,
            "sbc": sbc,
            "offc": offc,
            "scalec": scalec,
        })
    return in_maps


def kernel(x, weight, antiquant_scale, antiquant_offset, _trace=False):
    from concourse.bass_utils import run_bass_kernel_spmd

    nc = _get_nc()
    in_maps = _prep_inputs(x, weight, antiquant_scale, antiquant_offset)
    res = run_bass_kernel_spmd(
        nc, in_maps, core_ids=list(range(NCORES)), trace=_trace
    )
    out = np.empty((M, N), dtype=np.float32)
    for c in range(NCORES):
        outt = np.asarray(res.results[c]["outt"])      # [N, MSH] bf16
        out[c * MSH:(c + 1) * MSH, :] = outt.T.astype(np.float32)
    if _trace:
        _cache["last_result"] = res
    return out
